# revision 50
# baseline (speedup 1.0000x reference)
"""Trainium2 Bass kernel for nn_Classifier (segment mean-pool + tiny MLP head).

Pipeline (matches the jax reference):
  pooled[g] = mean of features over nodes with batch id g   (2048 graphs)
  out = LeakyReLU(LayerNorm(pooled @ W1 + b1)) @ W2 + b2    -> [2048, 1]

Sharding strategy: the batch ids are sorted, so nodes are split across the 8
cores at segment-block boundaries — core i owns graphs [256i, 256i+256) and
exactly the nodes belonging to them. Each core computes segment sums for its
own 256 graphs (disjoint), so no collective is needed; the host concatenates
the 8 per-core [256]-sized outputs.

Per core, segment sums are computed on the tensor engine: for each 128-node
sub-tile, a one-hot matrix [128 nodes, 128 segs] is used as the matmul
stationary against the node features [128, 256], accumulating into PSUM.

Design notes (vs a 561us fp32r baseline; now ~121us, DMA-roofline-bound):
  * features stream as fp16 — halves HBM traffic (the dominant cost; the
    stream runs at ~320 GB/s/core, the practical per-core DMA ceiling).
    The PE runs 16-bit matmuls at 1 cycle/row; sums accumulate in fp32
    PSUM, so only the input rounding (2^-11) is lost (rel err ~3e-4).
  * one-hot generation (the old bottleneck: tensor_scalar with a
    per-partition scalar operand hits the TensorScalarPtr slow path,
    ~1.2-2.2us/op) is a single tensor_tensor(is_equal) per 1024-node
    chunk with broadcast (stride-0) APs — and since batch ids are
    sorted, only the ~8 segment columns a chunk can touch are compared
    (host-computed window); the rest of the 128-wide stationary stays
    zero (only stale windows are re-zeroed).
  * per-segment reciprocal counts come from the host (it already scans
    `batch` for shard boundaries) — no ones column, no on-device counts.
  * feature chunks round-robin the three DMA queues (sync/scalar/pool)
    whole (4KB partition lines); the last few chunks are split across
    queues so all queues finish together and the in-order PE isn't left
    with a laggard-queue backlog.
  * the MLP head runs per 128-segment region as soon as that region's
    sums finish (region 0's head hides under region 1's stream), uses
    bn_stats/bn_aggr for LN moments, and writes the output through a
    PE transpose so the final DMA is one contiguous row, not 128
    scattered 4B descriptors.
  * when the actual inputs allow it (gamma > 0, beta/b1/b2 zero — checked
    at runtime, general fallback otherwise), LN's affine folds into W2
    and the bias matmuls are skipped.
"""

from contextlib import ExitStack

import numpy as np

import concourse.bass as bass
import concourse.mybir as mybir
import concourse.tile as tile
from concourse.bass_utils import run_bass_kernel_spmd

# ---------------------------------------------------------------------------
# Workaround: this walrus build rejects instructions carrying more than one
# semaphore wait ("Too many sync wait commands"), but Tile's semaphore
# assignment freely attaches several. After the TileContext has lowered the
# program, split any excess waits onto same-engine nops inserted right before
# the instruction (semantics are identical: all waits are monotonic and must
# hold before the instruction issues).
_MAX_WAITS = 1


def _split_excess_waits(nc: "bass.Bass", max_waits: int = _MAX_WAITS) -> None:
    ctr = 0
    for f in nc.m.functions:
        for b in f.blocks:
            out = []
            for inst in b.instructions:
                si = inst.sync_info
                waits = list(si.on_wait) if (si is not None and si.on_wait) else []
                if len(waits) > max_waits:
                    keep = waits[-max_waits:]
                    extra = waits[:-max_waits]
                    # On the PE queue the carrier must be a DRAIN: silicon
                    # promotes waitless LDWEIGHTS past in-flight work, so a
                    # plain nop's wait can be bypassed (walrus attaches a
                    # matmul's waits to its LDWEIGHTS — stripping them onto a
                    # nop re-opens that race). A drain fully serializes.
                    is_pe = inst.engine == mybir.EngineType.PE
                    for i in range(0, len(extra), max_waits):
                        ctr += 1
                        if is_pe:
                            nop = mybir.InstDrain(
                                name=f"waitsplit_drain_{ctr}", ins=[], outs=[],
                                engine=inst.engine,
                            )
                        else:
                            nop = mybir.InstNoOp(
                                name=f"waitsplit_nop_{ctr}", ins=[], outs=[],
                                engine=inst.engine,
                            )
                        nop.sync_info = mybir.SyncInfo(
                            on_wait=extra[i : i + max_waits], on_update=[]
                        )
                        nc.register_instruction(nop)
                        out.append(nop)
                    inst.sync_info = mybir.SyncInfo(
                        on_wait=keep, on_update=list(si.on_update or [])
                    )
                out.append(inst)
            b.instructions = out
# ---------------------------------------------------------------------------

N_CORES = 8
NUM_GRAPHS = 2048
SEGS_PER_CORE = NUM_GRAPHS // N_CORES  # 256
D = 256
K_SUB = 8  # 128-node sub-tiles per DMA chunk (chunk = 1024 nodes, 512 KB fp16)
CHUNK = 128 * K_SUB
LN_EPS = 1e-5
NEG_SLOPE = 0.01

_F32 = mybir.dt.float32
_F16 = mybir.dt.float16
_ALU = mybir.AluOpType

# Test/debug hooks: set PROFILE=True before calling kernel() to request an
# NTFF trace; the BassKernelResults lands in LAST_RESULT.
PROFILE = False
PROFILE_DIR = None
LAST_RESULT = None


def _build_program(
    chunks_per_region: int,
    W: int,
    bases: list[int],
    fold_affine: bool,
    skip_b1: bool,
    skip_b2: bool,
) -> bass.Bass:
    """W: one-hot window width (segments a 1024-node chunk can touch, padded);
    bases[chunk]: first segment id of chunk's window (same across cores).
    fold_affine: LN's gamma was folded into W2 on the host (valid when
    gamma > 0 and beta == 0, since lrelu is positive-homogeneous) — skip the
    gamma/beta ops. skip_b1/skip_b2: those biases are all-zero, skip them."""
    R = chunks_per_region
    C = 2 * R  # chunks per core (2 segment blocks of 128)
    n_nodes = C * CHUNK

    nc = bass.Bass("TRN2", debug=False)
    feat = nc.dram_tensor("feat", [n_nodes, D], _F16, kind="ExternalInput").ap()
    segT = nc.dram_tensor("segT", [128, C * K_SUB], _F16, kind="ExternalInput").ap()
    ident_d = nc.dram_tensor("ident", [128, 128], _F32, kind="ExternalInput").ap()
    w1aug_d = nc.dram_tensor("w1aug", [D + 1, 128], _F32, kind="ExternalInput").ap()
    pvec_d = nc.dram_tensor("pvec", [1, 385], _F32, kind="ExternalInput").ap()
    rcnt_d = nc.dram_tensor("rcnt", [128, 2], _F32, kind="ExternalInput").ap()
    out_d = nc.dram_tensor("out", [1, 256], _F32, kind="ExternalOutput").ap()

    with tile.TileContext(nc) as tc, ExitStack() as ctx:
        cpool = ctx.enter_context(tc.tile_pool(name="consts", bufs=1))
        fpool = ctx.enter_context(tc.tile_pool(name="feat", bufs=12))
        opool = ctx.enter_context(tc.tile_pool(name="oh", bufs=8))
        acc = ctx.enter_context(tc.tile_pool(name="acc", bufs=1, space="PSUM"))
        ppool = ctx.enter_context(tc.tile_pool(name="pw", bufs=1, space="PSUM"))
        spool = ctx.enter_context(tc.tile_pool(name="small", bufs=2))

        # the one-hot compare needs segT (DMA, gpsimd queue) and iota
        # (generated on the DVE — 128 tiny DMA lines would be slower);
        # the feature chunks then lead the sync/scalar queues.
        segT_t = cpool.tile([128, C * K_SUB], _F16, tag="segT")
        nc.sync.dma_start(out=segT_t[:], in_=segT[:])
        iota_t = cpool.tile([128, 128], _F16, tag="iota")
        nc.gpsimd.iota(
            iota_t[:], pattern=[[1, 128]], base=0, channel_multiplier=0,
            allow_small_or_imprecise_dtypes=True,
        )
        # head-only consts are DMA'd mid-stream (see the chunk loop below) so
        # the three DMA queues start on feature chunks immediately; tiles are
        # just allocated here.
        ident_t = cpool.tile([128, 128], _F32, tag="ident")
        w1a = cpool.tile([128, 128], _F32, tag="w1a")
        w1b = cpool.tile([128, 128], _F32, tag="w1b")
        w1c = cpool.tile([1, 128], _F32, tag="w1c")
        pv = cpool.tile([1, 385], _F32, tag="pv")
        rcnt_t = cpool.tile([128, 2], _F32, tag="rcnt")
        ones_row = cpool.tile([1, 128], _F32, tag="ones")
        out_sb = cpool.tile([1, 256], _F32, tag="outsb")
        epsc = cpool.tile([128, 1], _F32, tag="epsc")
        bc = cpool.tile([128, 385], _F32, tag="bcs")

        def _late_consts():
            # two small DMAs per queue, slotted behind each queue's first
            # feature chunks
            nc.scalar.dma_start(out=ident_t[:], in_=ident_d[:])
            nc.scalar.dma_start(out=w1a[:], in_=w1aug_d[0:128, :])
            nc.gpsimd.dma_start(out=w1b[:], in_=w1aug_d[128:256, :])
            nc.gpsimd.dma_start(out=w1c[:], in_=w1aug_d[256:257, :])
            nc.sync.dma_start(out=pv[:], in_=pvec_d[:])
            nc.sync.dma_start(out=rcnt_t[:], in_=rcnt_d[:])
            nc.vector.memset(ones_row[:], 1.0)
            nc.vector.memset(epsc[:], LN_EPS)

        # ---- main stream: per-segment sums, one 128-seg region at a time ----
        sums = [acc.tile([128, D], _F32, tag=f"sum{r}", name=f"sum{r}") for r in range(2)]
        dma_engs = (nc.sync, nc.scalar, nc.gpsimd)
        OH_BUFS = 8
        prev_win = {}  # opool buffer slot -> last hot window
        for r in range(2):
            for c in range(R):
                chunk = r * R + c
                if chunk == 3:
                    _late_consts()
                ft = fpool.tile([128, K_SUB, D], _F16, tag="ft")
                src = feat[chunk * CHUNK : (chunk + 1) * CHUNK, :].rearrange(
                    "(p k) f -> p k f", p=128
                )
                # whole chunks (4KB partition lines — best DMA efficiency)
                # round-robin the three queues; only the last few chunks are
                # split across two queues so the queues converge at stream
                # end instead of leaving the in-order PE a laggard's backlog
                if chunk < C - 6:
                    dma_engs[chunk % 3].dma_start(out=ft[:], in_=src)
                else:
                    h = K_SUB // 2
                    dma_engs[2 * chunk % 3].dma_start(
                        out=ft[:, 0:h, :], in_=src[:, 0:h, :]
                    )
                    dma_engs[(2 * chunk + 1) % 3].dma_start(
                        out=ft[:, h:K_SUB, :], in_=src[:, h:K_SUB, :]
                    )
                # The batch ids are sorted, so a chunk only touches segments
                # [bases[chunk], bases[chunk]+W) — the one-hot stationary is
                # kept 128 wide (PE out sub-windows have alignment limits)
                # but only the stale window is re-zeroed and only the hot
                # window compared: oh[p,k,s] = (iota[s] == segid[p, c*8+k])
                oh = opool.tile([128, K_SUB, 128], _F16, tag="oh")
                b = bases[chunk]
                slot = chunk % OH_BUFS
                if slot not in prev_win:
                    nc.vector.memset(oh[:], 0.0)
                else:
                    pb = prev_win[slot]
                    nc.vector.memset(oh[:, :, pb : pb + W], 0.0)
                prev_win[slot] = b
                in0 = iota_t[:, b : b + W].unsqueeze(1).broadcast_to([128, K_SUB, W])
                in1 = (
                    segT_t[:, chunk * K_SUB : (chunk + 1) * K_SUB]
                    .unsqueeze(2)
                    .broadcast_to([128, K_SUB, W])
                )
                nc.vector.tensor_tensor(
                    out=oh[:, :, b : b + W], in0=in0, in1=in1, op=_ALU.is_equal
                )
                for k in range(K_SUB):
                    nc.tensor.matmul(
                        out=sums[r][:],
                        lhsT=oh[:, k, :],
                        rhs=ft[:, k, :],
                        start=(c == 0 and k == 0),
                        stop=(c == R - 1 and k == K_SUB - 1),
                    )

            # ---- region tail: pooled mean, transpose, MLP head for this
            # region's 128 graphs (overlaps the next region's stream) ----
            if r == 0:
                # broadcast [gamma | beta | W2 | b2] to all 128 partitions
                bc_ps = ppool.tile([128, 385], _F32, tag="bc")
                nc.tensor.matmul(
                    out=bc_ps[:], lhsT=ones_row[:], rhs=pv[:], start=True, stop=True
                )
                nc.scalar.copy(bc[:], bc_ps[:])
            # pooled mean via the scalar engine's per-partition scale
            pooled = spool.tile([128, D], _F32, tag="pooled")
            nc.scalar.activation(
                pooled[:], sums[r][:], mybir.ActivationFunctionType.Copy,
                scale=rcnt_t[:, r : r + 1],
            )
            tpw = ppool.tile([128, D], _F32, tag="tpw")
            for fb in range(2):
                nc.tensor.transpose(
                    out=tpw[:, fb * 128 : (fb + 1) * 128],
                    in_=pooled[:, fb * 128 : (fb + 1) * 128],
                    identity=ident_t[:],
                )
            ptT = spool.tile([128, D], _F32, tag="ptT")
            nc.scalar.copy(ptT[:], tpw[:])

            # h = pooled @ W1 + b1; LayerNorm; LeakyReLU; @ W2 + b2
            h_ps = ppool.tile([128, 128], _F32, tag="h")
            nc.tensor.matmul(
                out=h_ps[:], lhsT=ptT[:, 0:128], rhs=w1a[:], start=True, stop=False
            )
            nc.tensor.matmul(
                out=h_ps[:], lhsT=ptT[:, 128:256], rhs=w1b[:], start=False,
                stop=skip_b1,
            )
            if not skip_b1:
                nc.tensor.matmul(
                    out=h_ps[:], lhsT=ones_row[:], rhs=w1c[:], start=False, stop=True
                )

            stats = spool.tile([128, 6], _F32, tag="stats")
            nc.vector.bn_stats(stats[:], h_ps[:])
            aggr = spool.tile([128, 2], _F32, tag="aggr")
            nc.vector.bn_aggr(aggr[:], stats[:])
            std = spool.tile([128, 1], _F32, tag="std")
            nc.scalar.activation(
                std[:], aggr[:, 1:2], mybir.ActivationFunctionType.Sqrt,
                bias=epsc[:], scale=1.0,
            )
            rstd = spool.tile([128, 1], _F32, tag="rstd")
            nc.vector.reciprocal(rstd[:], std[:])
            # yn = (h - mu) * rstd in one two-scalar op
            yn = spool.tile([128, 128], _F32, tag="yn")
            nc.vector.tensor_scalar(
                out=yn[:], in0=h_ps[:], scalar1=aggr[:, 0:1], scalar2=rstd[:],
                op0=_ALU.subtract, op1=_ALU.mult,
            )
            if fold_affine:
                y3 = yn
            else:
                y2 = spool.tile([128, 128], _F32, tag="y2")
                nc.vector.scalar_tensor_tensor(
                    out=y2[:], in0=yn[:], scalar=1.0, in1=bc[:, 0:128],
                    op0=_ALU.mult, op1=_ALU.mult,
                )
                y3 = spool.tile([128, 128], _F32, tag="y3")
                nc.vector.tensor_tensor(out=y3[:], in0=y2[:], in1=bc[:, 128:256],
                                        op=_ALU.add)
            yl = spool.tile([128, 128], _F32, tag="yl")
            nc.vector.scalar_tensor_tensor(
                out=yl[:], in0=y3[:], scalar=NEG_SLOPE, in1=y3[:],
                op0=_ALU.mult, op1=_ALU.max,
            )
            prod = spool.tile([128, 128], _F32, tag="prod")
            oc = spool.tile([128, 1], _F32, tag="oc")
            nc.vector.scalar_tensor_tensor(
                out=prod[:], in0=yl[:], scalar=1.0, in1=bc[:, 256:384],
                op0=_ALU.mult, op1=_ALU.mult, accum_out=oc[:],
            )
            # transpose [128,1] -> a contiguous [1,128] row so the final DMA
            # is one 512B line instead of 128 scattered 4B descriptors; the
            # b2 bias is added by a second accumulating matmul (bc's b2
            # column sums against ident's single 1 per output position)
            ot_ps = ppool.tile([1, 128], _F32, tag="ot")
            nc.tensor.matmul(
                out=ot_ps[:], lhsT=oc[:], rhs=ident_t[:], start=True, stop=skip_b2
            )
            if not skip_b2:
                nc.tensor.matmul(
                    out=ot_ps[:], lhsT=bc[:, 384:385], rhs=ident_t[:], start=False,
                    stop=True,
                )
            nc.scalar.copy(out_sb[:, r * 128 : (r + 1) * 128], ot_ps[:])
        nc.sync.dma_start(out=out_d[:], in_=out_sb[:])

    _split_excess_waits(nc)
    return nc


def _prep_inputs(features, batch):
    """Segment-block-aligned sharding + per-core padded fp16 arrays."""
    feats = np.asarray(features)
    seg = np.asarray(batch).astype(np.int64)
    counts = np.bincount(seg, minlength=NUM_GRAPHS)
    bnd = np.zeros(NUM_GRAPHS + 1, np.int64)
    bnd[1:] = np.cumsum(counts)

    block_lo = bnd[0 : NUM_GRAPHS : 128]
    block_hi = bnd[128 : NUM_GRAPHS + 1 : 128]
    block_n = block_hi - block_lo  # nodes per 128-segment block (16 blocks)
    R = int(np.max((block_n + CHUNK - 1) // CHUNK))  # chunks per region
    region = R * CHUNK
    ncap = 2 * region

    feat16 = np.zeros((N_CORES, ncap, D), np.float16)
    seg_adj = np.full((N_CORES, ncap), -1, np.int32)
    for i in range(N_CORES):
        for r in range(2):
            b = 2 * i + r
            lo, hi = int(block_lo[b]), int(block_hi[b])
            m = hi - lo
            off = r * region
            feat16[i, off : off + m, :] = feats[lo:hi]
            seg_adj[i, off : off + m] = seg[lo:hi] - 128 * b
    # the batch ids are sorted, so a 1024-node chunk only touches a narrow
    # band of segments. Compute a window [bases[c], bases[c]+W) per chunk
    # position (shared by all cores — SPMD needs identical programs); the
    # device compares only those W one-hot columns.
    ch = seg_adj.reshape(N_CORES, 2 * R, CHUNK)
    masked = np.ma.masked_less(ch, 0)
    cmin = masked.min(axis=2).filled(0).min(axis=0)  # [2R] per-chunk base
    cmax = masked.max(axis=2).filled(0).max(axis=0)
    W = max(8, int((cmax - cmin).max()) + 1)
    assert W <= 128, W
    bases = np.minimum(cmin, 128 - W).astype(np.int64)
    # transpose seg ids to match the on-chip [partition, sub-tile] layout:
    # node (chunk*1024 + p*8 + k) -> segT[p, chunk*8 + k]
    segT = (
        seg_adj.astype(np.float16)
        .reshape(N_CORES, -1, 128, K_SUB)
        .transpose(0, 2, 1, 3)
        .reshape(N_CORES, 128, -1)
    )
    # reciprocal counts: rcnt[i, p, r] = 1 / max(count[graph 128*(2i+r)+p], 1)
    rc = (1.0 / np.maximum(counts, 1)).astype(np.float32).reshape(16, 128)
    rcnt = np.stack(
        [np.stack([rc[2 * i], rc[2 * i + 1]], axis=1) for i in range(N_CORES)]
    )
    return feat16, np.ascontiguousarray(segT), rcnt, R, W, [int(b) for b in bases]


def kernel(features, batch, W1, b1, gamma, beta, W2, b2):
    feat16, segT, rcnt, R, W, bases = _prep_inputs(features, batch)

    ident = np.eye(128, dtype=np.float32)
    w1aug = np.concatenate(
        [np.asarray(W1, np.float32), np.asarray(b1, np.float32)[None, :]], axis=0
    )
    gamma_f = np.asarray(gamma, np.float32).ravel()
    beta_f = np.asarray(beta, np.float32).ravel()
    w2_f = np.asarray(W2, np.float32).ravel()
    b2_f = np.asarray(b2, np.float32).ravel()
    # lrelu(g*x) == g*lrelu(x) for g > 0, so LN's affine folds into W2 when
    # beta is zero; all-zero biases skip their ops outright
    fold_affine = bool(np.all(gamma_f > 0) and np.all(beta_f == 0.0))
    skip_b1 = bool(np.all(np.asarray(b1, np.float32) == 0.0))
    skip_b2 = bool(np.all(b2_f == 0.0))
    w2_eff = w2_f * gamma_f if fold_affine else w2_f
    pvec = np.concatenate([gamma_f, beta_f, w2_eff, b2_f])[None, :]

    nc = _build_program(R, W, bases, fold_affine, skip_b1, skip_b2)
    in_maps = [
        {
            "feat": feat16[i],
            "segT": segT[i],
            "ident": ident,
            "w1aug": w1aug,
            "pvec": pvec,
            "rcnt": rcnt[i],
        }
        for i in range(N_CORES)
    ]
    res = run_bass_kernel_spmd(
        nc, in_maps, list(range(N_CORES)), trace=PROFILE, tmpdir=PROFILE_DIR
    )
    global LAST_RESULT
    LAST_RESULT = res
    out = np.concatenate(
        [res.results[i]["out"].reshape(SEGS_PER_CORE) for i in range(N_CORES)]
    )
    return out.reshape(NUM_GRAPHS, 1).astype(np.float32)


# revision 52
# speedup vs baseline: 1.0269x; 1.0269x over previous
"""Trainium2 Bass kernel for nn_Classifier (segment mean-pool + tiny MLP head).

Pipeline (matches the jax reference):
  pooled[g] = mean of features over nodes with batch id g   (2048 graphs)
  out = LeakyReLU(LayerNorm(pooled @ W1 + b1)) @ W2 + b2    -> [2048, 1]

Sharding strategy: the batch ids are sorted, so nodes are split across the 8
cores at segment-block boundaries — core i owns graphs [256i, 256i+256) and
exactly the nodes belonging to them. Each core computes segment sums for its
own 256 graphs (disjoint), so no collective is needed; the host concatenates
the 8 per-core [256]-sized outputs.

Per core, segment sums are computed on the tensor engine: for each 128-node
sub-tile, a one-hot matrix [128 nodes, 128 segs] is used as the matmul
stationary against the node features [128, 256], accumulating into PSUM.

Design notes (vs a 561us fp32r baseline; now ~121us, DMA-roofline-bound):
  * features stream as fp16 — halves HBM traffic (the dominant cost; the
    stream runs at ~320 GB/s/core, the practical per-core DMA ceiling).
    The PE runs 16-bit matmuls at 1 cycle/row; sums accumulate in fp32
    PSUM, so only the input rounding (2^-11) is lost (rel err ~3e-4).
  * one-hot generation (the old bottleneck: tensor_scalar with a
    per-partition scalar operand hits the TensorScalarPtr slow path,
    ~1.2-2.2us/op) is a single tensor_tensor(is_equal) per 1024-node
    chunk with broadcast (stride-0) APs — and since batch ids are
    sorted, only the ~8 segment columns a chunk can touch are compared
    (host-computed window); the rest of the 128-wide stationary stays
    zero (only stale windows are re-zeroed).
  * per-segment reciprocal counts come from the host (it already scans
    `batch` for shard boundaries) — no ones column, no on-device counts.
  * feature chunks round-robin the three DMA queues (sync/scalar/pool)
    whole (4KB partition lines); the last few chunks are split across
    queues so all queues finish together and the in-order PE isn't left
    with a laggard-queue backlog.
  * the MLP head runs per 128-segment region as soon as that region's
    sums finish (region 0's head hides under region 1's stream), uses
    bn_stats/bn_aggr for LN moments, and writes the output through a
    PE transpose so the final DMA is one contiguous row, not 128
    scattered 4B descriptors.
  * when the actual inputs allow it (gamma > 0, beta/b1/b2 zero — checked
    at runtime, general fallback otherwise), LN's affine folds into W2
    and the bias matmuls are skipped.
"""

from contextlib import ExitStack

import numpy as np

import concourse.bass as bass
import concourse.mybir as mybir
import concourse.tile as tile
from concourse.bass_utils import run_bass_kernel_spmd

# ---------------------------------------------------------------------------
# Workaround: this walrus build rejects instructions carrying more than one
# semaphore wait ("Too many sync wait commands"), but Tile's semaphore
# assignment freely attaches several. After the TileContext has lowered the
# program, split any excess waits onto same-engine nops inserted right before
# the instruction (semantics are identical: all waits are monotonic and must
# hold before the instruction issues).
_MAX_WAITS = 1


def _split_excess_waits(nc: "bass.Bass", max_waits: int = _MAX_WAITS) -> None:
    ctr = 0
    for f in nc.m.functions:
        for b in f.blocks:
            out = []
            for inst in b.instructions:
                si = inst.sync_info
                waits = list(si.on_wait) if (si is not None and si.on_wait) else []
                if len(waits) > max_waits:
                    keep = waits[-max_waits:]
                    extra = waits[:-max_waits]
                    # On the PE queue the carrier must be a DRAIN: silicon
                    # promotes waitless LDWEIGHTS past in-flight work, so a
                    # plain nop's wait can be bypassed (walrus attaches a
                    # matmul's waits to its LDWEIGHTS — stripping them onto a
                    # nop re-opens that race). A drain fully serializes.
                    is_pe = inst.engine == mybir.EngineType.PE
                    for i in range(0, len(extra), max_waits):
                        ctr += 1
                        if is_pe:
                            nop = mybir.InstDrain(
                                name=f"waitsplit_drain_{ctr}", ins=[], outs=[],
                                engine=inst.engine,
                            )
                        else:
                            nop = mybir.InstNoOp(
                                name=f"waitsplit_nop_{ctr}", ins=[], outs=[],
                                engine=inst.engine,
                            )
                        nop.sync_info = mybir.SyncInfo(
                            on_wait=extra[i : i + max_waits], on_update=[]
                        )
                        nc.register_instruction(nop)
                        out.append(nop)
                    inst.sync_info = mybir.SyncInfo(
                        on_wait=keep, on_update=list(si.on_update or [])
                    )
                out.append(inst)
            b.instructions = out
# ---------------------------------------------------------------------------

N_CORES = 8
NUM_GRAPHS = 2048
SEGS_PER_CORE = NUM_GRAPHS // N_CORES  # 256
D = 256
K_SUB = 8  # 128-node sub-tiles per DMA chunk (chunk = 1024 nodes, 512 KB fp16)
CHUNK = 128 * K_SUB
LN_EPS = 1e-5
NEG_SLOPE = 0.01

_F32 = mybir.dt.float32
_F16 = mybir.dt.float16
_ALU = mybir.AluOpType

# Test/debug hooks: set PROFILE=True before calling kernel() to request an
# NTFF trace; the BassKernelResults lands in LAST_RESULT.
PROFILE = False
PROFILE_DIR = None
LAST_RESULT = None


def _build_program(
    chunks_per_region: int,
    W: int,
    bases: list[int],
    fold_affine: bool,
    skip_b1: bool,
    skip_b2: bool,
) -> bass.Bass:
    """W: one-hot window width (segments a 1024-node chunk can touch, padded);
    bases[chunk]: first segment id of chunk's window (same across cores).
    fold_affine: LN's gamma was folded into W2 on the host (valid when
    gamma > 0 and beta == 0, since lrelu is positive-homogeneous) — skip the
    gamma/beta ops. skip_b1/skip_b2: those biases are all-zero, skip them."""
    R = chunks_per_region
    C = 2 * R  # chunks per core (2 segment blocks of 128)
    n_nodes = C * CHUNK

    nc = bass.Bass("TRN2", debug=False)
    feat = nc.dram_tensor("feat", [n_nodes, D], _F16, kind="ExternalInput").ap()
    segT = nc.dram_tensor("segT", [128, C * K_SUB], _F16, kind="ExternalInput").ap()
    ident_d = nc.dram_tensor("ident", [128, 128], _F32, kind="ExternalInput").ap()
    w1aug_d = nc.dram_tensor("w1aug", [D + 1, 128], _F32, kind="ExternalInput").ap()
    pvec_d = nc.dram_tensor("pvec", [1, 385], _F32, kind="ExternalInput").ap()
    rcnt_d = nc.dram_tensor("rcnt", [128, 2], _F32, kind="ExternalInput").ap()
    out_d = nc.dram_tensor("out", [1, 256], _F32, kind="ExternalOutput").ap()

    with tile.TileContext(nc) as tc, ExitStack() as ctx:
        cpool = ctx.enter_context(tc.tile_pool(name="consts", bufs=1))
        fpool = ctx.enter_context(tc.tile_pool(name="feat", bufs=12))
        opool = ctx.enter_context(tc.tile_pool(name="oh", bufs=8))
        acc = ctx.enter_context(tc.tile_pool(name="acc", bufs=1, space="PSUM"))
        ppool = ctx.enter_context(tc.tile_pool(name="pw", bufs=1, space="PSUM"))
        spool = ctx.enter_context(tc.tile_pool(name="small", bufs=2))

        # the one-hot compare needs segT (DMA, gpsimd queue) and iota
        # (generated on the DVE — 128 tiny DMA lines would be slower);
        # the feature chunks then lead the sync/scalar queues.
        segT_t = cpool.tile([128, C * K_SUB], _F16, tag="segT")
        nc.sync.dma_start(out=segT_t[:], in_=segT[:])
        iota_t = cpool.tile([128, 128], _F16, tag="iota")
        nc.gpsimd.iota(
            iota_t[:], pattern=[[1, 128]], base=0, channel_multiplier=0,
            allow_small_or_imprecise_dtypes=True,
        )
        # head-only consts are DMA'd mid-stream (see the chunk loop below) so
        # the three DMA queues start on feature chunks immediately; tiles are
        # just allocated here.
        ident_t = cpool.tile([128, 128], _F32, tag="ident")
        w1a = cpool.tile([128, 128], _F32, tag="w1a")
        w1b = cpool.tile([128, 128], _F32, tag="w1b")
        w1c = cpool.tile([1, 128], _F32, tag="w1c")
        pv = cpool.tile([1, 385], _F32, tag="pv")
        rcnt_t = cpool.tile([128, 2], _F32, tag="rcnt")
        ones_row = cpool.tile([1, 128], _F32, tag="ones")
        out_sb = cpool.tile([1, 256], _F32, tag="outsb")
        epsc = cpool.tile([128, 1], _F32, tag="epsc")
        bc = cpool.tile([128, 385], _F32, tag="bcs")

        def _late_consts():
            # two small DMAs per queue, slotted behind each queue's first
            # feature chunks
            nc.scalar.dma_start(out=ident_t[:], in_=ident_d[:])
            nc.scalar.dma_start(out=w1a[:], in_=w1aug_d[0:128, :])
            nc.gpsimd.dma_start(out=w1b[:], in_=w1aug_d[128:256, :])
            nc.gpsimd.dma_start(out=w1c[:], in_=w1aug_d[256:257, :])
            nc.sync.dma_start(out=pv[:], in_=pvec_d[:])
            nc.sync.dma_start(out=rcnt_t[:], in_=rcnt_d[:])
            nc.vector.memset(ones_row[:], 1.0)
            nc.vector.memset(epsc[:], LN_EPS)

        # ---- main stream: per-segment sums, one 128-seg region at a time ----
        sums = [acc.tile([128, D], _F32, tag=f"sum{r}", name=f"sum{r}") for r in range(2)]
        dma_engs = (nc.sync, nc.scalar, nc.gpsimd)
        OH_BUFS = 8
        prev_win = {}  # opool buffer slot -> last hot window
        for r in range(2):
            for c in range(R):
                chunk = r * R + c
                if chunk == 3:
                    _late_consts()
                ft = fpool.tile([128, K_SUB, D], _F16, tag="ft")
                src = feat[chunk * CHUNK : (chunk + 1) * CHUNK, :].rearrange(
                    "(p k) f -> p k f", p=128
                )
                # whole chunks (4KB partition lines — best DMA efficiency)
                # round-robin the three queues; only the last few chunks are
                # split across two queues so the queues converge at stream
                # end instead of leaving the in-order PE a laggard's backlog
                if chunk < C - 6:
                    dma_engs[chunk % 3].dma_start(out=ft[:], in_=src)
                else:
                    h = K_SUB // 2
                    dma_engs[2 * chunk % 3].dma_start(
                        out=ft[:, 0:h, :], in_=src[:, 0:h, :]
                    )
                    dma_engs[(2 * chunk + 1) % 3].dma_start(
                        out=ft[:, h:K_SUB, :], in_=src[:, h:K_SUB, :]
                    )
                # The batch ids are sorted, so a chunk only touches segments
                # [bases[chunk], bases[chunk]+W) — the one-hot stationary is
                # kept 128 wide (PE out sub-windows have alignment limits)
                # but only the stale window is re-zeroed and only the hot
                # window compared: oh[p,k,s] = (iota[s] == segid[p, c*8+k])
                oh = opool.tile([128, K_SUB, 128], _F16, tag="oh")
                b = bases[chunk]
                slot = chunk % OH_BUFS
                if slot not in prev_win:
                    nc.vector.memset(oh[:], 0.0)
                else:
                    pb = prev_win[slot]
                    nc.vector.memset(oh[:, :, pb : pb + W], 0.0)
                prev_win[slot] = b
                in0 = iota_t[:, b : b + W].unsqueeze(1).broadcast_to([128, K_SUB, W])
                in1 = (
                    segT_t[:, chunk * K_SUB : (chunk + 1) * K_SUB]
                    .unsqueeze(2)
                    .broadcast_to([128, K_SUB, W])
                )
                nc.vector.tensor_tensor(
                    out=oh[:, :, b : b + W], in0=in0, in1=in1, op=_ALU.is_equal
                )
                for k in range(K_SUB):
                    nc.tensor.matmul(
                        out=sums[r][:],
                        lhsT=oh[:, k, :],
                        rhs=ft[:, k, :],
                        start=(c == 0 and k == 0),
                        stop=(c == R - 1 and k == K_SUB - 1),
                    )

            # ---- region tail: pooled mean, transpose, MLP head for this
            # region's 128 graphs (overlaps the next region's stream) ----
            if r == 0:
                # broadcast [gamma | beta | W2 | b2] to all 128 partitions
                bc_ps = ppool.tile([128, 385], _F32, tag="bc")
                nc.tensor.matmul(
                    out=bc_ps[:], lhsT=ones_row[:], rhs=pv[:], start=True, stop=True
                )
                nc.scalar.copy(bc[:], bc_ps[:])
            # pooled mean via the scalar engine's per-partition scale
            pooled = spool.tile([128, D], _F32, tag="pooled")
            nc.scalar.activation(
                pooled[:], sums[r][:], mybir.ActivationFunctionType.Copy,
                scale=rcnt_t[:, r : r + 1],
            )
            tpw = ppool.tile([128, D], _F32, tag="tpw")
            for fb in range(2):
                nc.tensor.transpose(
                    out=tpw[:, fb * 128 : (fb + 1) * 128],
                    in_=pooled[:, fb * 128 : (fb + 1) * 128],
                    identity=ident_t[:],
                )
            ptT = spool.tile([128, D], _F32, tag="ptT")
            nc.scalar.copy(ptT[:], tpw[:])

            # h = pooled @ W1 + b1; LayerNorm; LeakyReLU; @ W2 + b2
            h_ps = ppool.tile([128, 128], _F32, tag="h")
            nc.tensor.matmul(
                out=h_ps[:], lhsT=ptT[:, 0:128], rhs=w1a[:], start=True, stop=False
            )
            nc.tensor.matmul(
                out=h_ps[:], lhsT=ptT[:, 128:256], rhs=w1b[:], start=False,
                stop=skip_b1,
            )
            if not skip_b1:
                nc.tensor.matmul(
                    out=h_ps[:], lhsT=ones_row[:], rhs=w1c[:], start=False, stop=True
                )

            stats = spool.tile([128, 6], _F32, tag="stats")
            nc.vector.bn_stats(stats[:], h_ps[:])
            aggr = spool.tile([128, 2], _F32, tag="aggr")
            nc.vector.bn_aggr(aggr[:], stats[:])
            std = spool.tile([128, 1], _F32, tag="std")
            nc.scalar.activation(
                std[:], aggr[:, 1:2], mybir.ActivationFunctionType.Sqrt,
                bias=epsc[:], scale=1.0,
            )
            rstd = spool.tile([128, 1], _F32, tag="rstd")
            oc = spool.tile([128, 1], _F32, tag="oc")
            if fold_affine:
                # rstd > 0 always, and lrelu(rstd*v) == rstd*lrelu(v), so
                # pull rstd out of the whole lrelu+dot chain and apply it to
                # the [128,1] result — the Sqrt's cross-engine round-trip
                # then overlaps the main DVE ops instead of gating them
                yn = spool.tile([128, 128], _F32, tag="yn")
                nc.vector.tensor_scalar(
                    out=yn[:], in0=h_ps[:], scalar1=aggr[:, 0:1], scalar2=None,
                    op0=_ALU.subtract,
                )
                yl = spool.tile([128, 128], _F32, tag="yl")
                nc.vector.scalar_tensor_tensor(
                    out=yl[:], in0=yn[:], scalar=NEG_SLOPE, in1=yn[:],
                    op0=_ALU.mult, op1=_ALU.max,
                )
                prod = spool.tile([128, 128], _F32, tag="prod")
                ocr = spool.tile([128, 1], _F32, tag="ocr")
                nc.vector.scalar_tensor_tensor(
                    out=prod[:], in0=yl[:], scalar=1.0, in1=bc[:, 256:384],
                    op0=_ALU.mult, op1=_ALU.mult, accum_out=ocr[:],
                )
                nc.vector.reciprocal(rstd[:], std[:])
                nc.vector.tensor_scalar(
                    out=oc[:], in0=ocr[:], scalar1=rstd[:], scalar2=None,
                    op0=_ALU.mult,
                )
            else:
                nc.vector.reciprocal(rstd[:], std[:])
                yn = spool.tile([128, 128], _F32, tag="yn")
                nc.vector.tensor_scalar(
                    out=yn[:], in0=h_ps[:], scalar1=aggr[:, 0:1], scalar2=rstd[:],
                    op0=_ALU.subtract, op1=_ALU.mult,
                )
                y2 = spool.tile([128, 128], _F32, tag="y2")
                nc.vector.scalar_tensor_tensor(
                    out=y2[:], in0=yn[:], scalar=1.0, in1=bc[:, 0:128],
                    op0=_ALU.mult, op1=_ALU.mult,
                )
                y3 = spool.tile([128, 128], _F32, tag="y3")
                nc.vector.tensor_tensor(out=y3[:], in0=y2[:], in1=bc[:, 128:256],
                                        op=_ALU.add)
                yl = spool.tile([128, 128], _F32, tag="yl")
                nc.vector.scalar_tensor_tensor(
                    out=yl[:], in0=y3[:], scalar=NEG_SLOPE, in1=y3[:],
                    op0=_ALU.mult, op1=_ALU.max,
                )
                prod = spool.tile([128, 128], _F32, tag="prod")
                nc.vector.scalar_tensor_tensor(
                    out=prod[:], in0=yl[:], scalar=1.0, in1=bc[:, 256:384],
                    op0=_ALU.mult, op1=_ALU.mult, accum_out=oc[:],
                )
            # transpose [128,1] -> a contiguous [1,128] row so the final DMA
            # is one 512B line instead of 128 scattered 4B descriptors; the
            # b2 bias is added by a second accumulating matmul (bc's b2
            # column sums against ident's single 1 per output position)
            ot_ps = ppool.tile([1, 128], _F32, tag="ot")
            nc.tensor.matmul(
                out=ot_ps[:], lhsT=oc[:], rhs=ident_t[:], start=True, stop=skip_b2
            )
            if not skip_b2:
                nc.tensor.matmul(
                    out=ot_ps[:], lhsT=bc[:, 384:385], rhs=ident_t[:], start=False,
                    stop=True,
                )
            nc.vector.tensor_copy(out_sb[:, r * 128 : (r + 1) * 128], ot_ps[:])
        nc.sync.dma_start(out=out_d[:], in_=out_sb[:])

    _split_excess_waits(nc)
    return nc


def _prep_inputs(features, batch):
    """Segment-block-aligned sharding + per-core padded fp16 arrays."""
    feats = np.asarray(features)
    seg = np.asarray(batch).astype(np.int64)
    counts = np.bincount(seg, minlength=NUM_GRAPHS)
    bnd = np.zeros(NUM_GRAPHS + 1, np.int64)
    bnd[1:] = np.cumsum(counts)

    block_lo = bnd[0 : NUM_GRAPHS : 128]
    block_hi = bnd[128 : NUM_GRAPHS + 1 : 128]
    block_n = block_hi - block_lo  # nodes per 128-segment block (16 blocks)
    R = int(np.max((block_n + CHUNK - 1) // CHUNK))  # chunks per region
    region = R * CHUNK
    ncap = 2 * region

    feat16 = np.zeros((N_CORES, ncap, D), np.float16)
    seg_adj = np.full((N_CORES, ncap), -1, np.int32)
    for i in range(N_CORES):
        for r in range(2):
            b = 2 * i + r
            lo, hi = int(block_lo[b]), int(block_hi[b])
            m = hi - lo
            off = r * region
            feat16[i, off : off + m, :] = feats[lo:hi]
            seg_adj[i, off : off + m] = seg[lo:hi] - 128 * b
    # the batch ids are sorted, so a 1024-node chunk only touches a narrow
    # band of segments. Compute a window [bases[c], bases[c]+W) per chunk
    # position (shared by all cores — SPMD needs identical programs); the
    # device compares only those W one-hot columns.
    ch = seg_adj.reshape(N_CORES, 2 * R, CHUNK)
    masked = np.ma.masked_less(ch, 0)
    cmin = masked.min(axis=2).filled(0).min(axis=0)  # [2R] per-chunk base
    cmax = masked.max(axis=2).filled(0).max(axis=0)
    W = max(8, int((cmax - cmin).max()) + 1)
    assert W <= 128, W
    bases = np.minimum(cmin, 128 - W).astype(np.int64)
    # transpose seg ids to match the on-chip [partition, sub-tile] layout:
    # node (chunk*1024 + p*8 + k) -> segT[p, chunk*8 + k]
    segT = (
        seg_adj.astype(np.float16)
        .reshape(N_CORES, -1, 128, K_SUB)
        .transpose(0, 2, 1, 3)
        .reshape(N_CORES, 128, -1)
    )
    # reciprocal counts: rcnt[i, p, r] = 1 / max(count[graph 128*(2i+r)+p], 1)
    rc = (1.0 / np.maximum(counts, 1)).astype(np.float32).reshape(16, 128)
    rcnt = np.stack(
        [np.stack([rc[2 * i], rc[2 * i + 1]], axis=1) for i in range(N_CORES)]
    )
    return feat16, np.ascontiguousarray(segT), rcnt, R, W, [int(b) for b in bases]


def kernel(features, batch, W1, b1, gamma, beta, W2, b2):
    feat16, segT, rcnt, R, W, bases = _prep_inputs(features, batch)

    ident = np.eye(128, dtype=np.float32)
    w1aug = np.concatenate(
        [np.asarray(W1, np.float32), np.asarray(b1, np.float32)[None, :]], axis=0
    )
    gamma_f = np.asarray(gamma, np.float32).ravel()
    beta_f = np.asarray(beta, np.float32).ravel()
    w2_f = np.asarray(W2, np.float32).ravel()
    b2_f = np.asarray(b2, np.float32).ravel()
    # lrelu(g*x) == g*lrelu(x) for g > 0, so LN's affine folds into W2 when
    # beta is zero; all-zero biases skip their ops outright
    fold_affine = bool(np.all(gamma_f > 0) and np.all(beta_f == 0.0))
    skip_b1 = bool(np.all(np.asarray(b1, np.float32) == 0.0))
    skip_b2 = bool(np.all(b2_f == 0.0))
    w2_eff = w2_f * gamma_f if fold_affine else w2_f
    pvec = np.concatenate([gamma_f, beta_f, w2_eff, b2_f])[None, :]

    nc = _build_program(R, W, bases, fold_affine, skip_b1, skip_b2)
    in_maps = [
        {
            "feat": feat16[i],
            "segT": segT[i],
            "ident": ident,
            "w1aug": w1aug,
            "pvec": pvec,
            "rcnt": rcnt[i],
        }
        for i in range(N_CORES)
    ]
    res = run_bass_kernel_spmd(
        nc, in_maps, list(range(N_CORES)), trace=PROFILE, tmpdir=PROFILE_DIR
    )
    global LAST_RESULT
    LAST_RESULT = res
    out = np.concatenate(
        [res.results[i]["out"].reshape(SEGS_PER_CORE) for i in range(N_CORES)]
    )
    return out.reshape(NUM_GRAPHS, 1).astype(np.float32)
